# revision 18
# baseline (speedup 1.0000x reference)
"""CvtSelfAttention TRN2 Bass kernel (v2: fp16 PE path).

Strategy (8 NeuronCores, batch data-parallel, 4 batch elems per core):

Device (per batch elem; all matmuls fp16 in / fp32 PSUM accum):
  1. Depthwise 3x3 convs (q: stride 1, k/v: stride 2) as diagonal-weight
     matmuls on the PE; per (c-tile, tap) the diag weights (BN scale folded)
     are preloaded ONCE into SBUF (host-precomputed fp16), taps accumulate
     in PSUM. BN shift is folded into the eviction.
  2. QKV linear projections as plain matmuls (contraction over embed):
       QT[e, i] (pixel queries, +b_q at eviction), KT[e, j] (NO k-bias:
       it cancels in softmax, the cls key is adjusted on host), V_aug
       [j, 12*(64+1)] (NO v-bias: added on host; ones-column per head
       yields softmax denominators during the ctx matmul).
  3. Per head: scoresT[j, i] = KT_h^T-slices @ QT_h (k=64, head pairs on
     PE row groups 0/64), exp via ACT (scores small, no max-subtraction),
     ctxT_aug[65, i] = V_aug_h^T @ expT. Software-pipelined one head deep
     so scores(h+1) fills the PE while ACT computes exp(h).
  4. cls-KEY scores for all 12 heads in one stuffed block-diagonal matmul.
Host (numpy, exact fp32):
  - layout prep (padded CHW transpose, BN folding, fp16 casts)
  - cls-query attention row (conv K/V rows DMA'd back)
  - merge cls-key term into ctx, add v-bias, final [B, L, E] assembly.
"""
import os
import sys
import numpy as np

for _p in ("/opt/trn_rl_repo", "/root/.axon_site/_ro/trn_rl_repo"):
    if os.path.isdir(_p) and _p not in sys.path:
        sys.path.append(_p)

import concourse.bass as bass
import concourse.bacc as bacc
import concourse.tile as tile
from concourse import mybir
from concourse.bass_utils import run_bass_kernel_spmd

EMBED = 768
HEADS = 12
D = 64
EPS = 1e-5
NCORES = 8
B_TOTAL = 32
NB = B_TOTAL // NCORES          # batch elems per core
CT = EMBED // 128               # 6 c-tiles
NPIX = 1024                     # stride-1 conv output pixels (i-dim on device)
NKV = 256                       # stride-2 conv output pixels (j-dim on device)
SM_SCALE = float(EMBED) ** -0.5

F32 = mybir.dt.float32
F16 = mybir.dt.float16
F8 = mybir.dt.float8e4

# set by kernel() for test harnesses to inspect
last_results = None


def _build_program(repeat: int = 1):
    nc = bacc.Bacc(None, target_bir_lowering=False, debug=False)

    # ---- DRAM I/O (per core) ----
    xpad = nc.dram_tensor("xpad", [NB, CT, 128, 1156], F16, kind="ExternalInput")
    # host-precomputed diagonal conv weights [128, ct, cv(q,v), tap, 128] fp16
    d128 = nc.dram_tensor("d128", [128, CT * 2 * 9 * 128], F16,
                          kind="ExternalInput")
    w_t = nc.dram_tensor("w_t", [3, CT, 128, EMBED], F16, kind="ExternalInput")
    wcolk = nc.dram_tensor("wcolk", [128, CT * 9], F32, kind="ExternalInput")
    shifts = nc.dram_tensor("shifts", [128, 3 * CT], F32, kind="ExternalInput")
    biasq = nc.dram_tensor("biasq", [128, CT], F32, kind="ExternalInput")
    kcls = nc.dram_tensor("kcls", [NB, 128, CT * HEADS], F16, kind="ExternalInput")

    ctxu = nc.dram_tensor("ctxu", [NB, HEADS, D + 1, NPIX], F16,
                          kind="ExternalOutput")
    ecls = nc.dram_tensor("ecls", [NB, HEADS, NPIX], F16, kind="ExternalOutput")
    kt_out = nc.dram_tensor("kt_out", [NB, 128, CT, NKV], F16,
                            kind="ExternalOutput")
    v_out = nc.dram_tensor("v_out", [NB, 128, 2, HEADS * D], F16,
                           kind="ExternalOutput")

    with tile.TileContext(nc) as tc:
        import contextlib
        with contextlib.ExitStack() as ctx:
            consts = ctx.enter_context(tc.tile_pool(name="consts", bufs=1))
            kclsp = ctx.enter_context(tc.tile_pool(name="kclsp", bufs=2))
            xpool = ctx.enter_context(tc.tile_pool(name="xpool", bufs=3))
            kaccp = ctx.enter_context(tc.tile_pool(name="kaccp", bufs=2))
            big = ctx.enter_context(tc.tile_pool(name="big", bufs=2))
            expp = ctx.enter_context(tc.tile_pool(name="expp", bufs=6))
            stage = ctx.enter_context(tc.tile_pool(name="stage", bufs=4))
            eclsp = ctx.enter_context(tc.tile_pool(name="eclsp", bufs=2))
            ps5 = ctx.enter_context(tc.tile_pool(name="ps5", bufs=2, space="PSUM"))
            ps10 = ctx.enter_context(tc.tile_pool(name="ps10", bufs=2, space="PSUM"))
            psctx = ctx.enter_context(tc.tile_pool(name="psctx", bufs=2, space="PSUM"))

            # ---- constants ----
            # startup-critical consts first: the first c-tile's diag weights
            # (split across queues), BN shifts, k tap scalars. Everything else
            # is DMA'd after conv(b0) is emitted so the xpad transfers win
            # the DMA queues.
            d128_sb = consts.tile([128, CT * 2 * 9 * 128], F16)
            d128_v = d128_sb.rearrange("p (t c k s) -> p t c k s", t=CT, c=2,
                                       k=9)
            d128_dv = d128[:, :].rearrange("p (t c k s) -> p t c k s", t=CT,
                                           c=2, k=9)
            for kk in range(3):
                nc.sync.dma_start(d128_v[:, 0, 0, 3 * kk:3 * kk + 3],
                                  d128_dv[:, 0, 0, 3 * kk:3 * kk + 3])
            shifts_sb = consts.tile([128, 3 * CT], F32)
            shifts_v = shifts_sb.rearrange("p (c t) -> p c t", c=3)
            wcolk_sb = consts.tile([128, CT * 9], F32)
            wcolk_v = wcolk_sb.rearrange("p (t k) -> p t k", t=CT)
            biasq_sb = consts.tile([128, CT], F32)
            wq = [consts.tile([128, EMBED], F16, name=f"wq{i}") for i in range(CT)]
            wk = [consts.tile([128, EMBED], F16, name=f"wk{i}") for i in range(CT)]
            wv = [consts.tile([128, EMBED], F16, name=f"wv{i}") for i in range(CT)]

            def conv_make(b, prefetch_d128=False):
                """Allocate b's conv tiles; return (tiles, generator) where the
                generator emits one PSUM group (9 diag matmuls + eviction) per
                next() so conv(b) can interleave into attention(b-1)."""
                convq = big.tile([128, CT, NPIX], F16, tag="convq")
                convk = big.tile([128, CT, NKV], F16, tag="convk")
                convv = big.tile([128, CT, NKV], F16, tag="convv")

                def gen():
                    for ct in range(CT):
                        if prefetch_d128 and ct + 1 < CT:
                            # b0 only: pull in the next c-tile's diag weights
                            # (must precede their first use in program order)
                            nc.sync.dma_start(d128_v[:, ct + 1],
                                              d128_dv[:, ct + 1])
                        # 1164 = 1156 + slack so tap-shifted row views stay
                        # in-bounds (their trailing cols are never read)
                        xp = xpool.tile([128, 1164], F16)
                        if prefetch_d128:
                            nc.sync.dma_start(xp[:, 0:616], xpad[b, ct][:, 0:616])
                            nc.sync.dma_start(xp[:, 616:1156],
                                              xpad[b, ct][:, 616:1156])
                            if ct == 0:
                                nc.sync.dma_start(shifts_sb, shifts[:, :])
                                nc.sync.dma_start(wcolk_sb, wcolk[:, :])
                                nc.sync.dma_start(d128_v[:, 0, 1],
                                                  d128_dv[:, 0, 1])
                        else:
                            nc.sync.dma_start(xp[:, 0:1156], xpad[b, ct])
                        # q (2 chunks) and v on the PE as fp16 diag matmuls
                        for cv, ch in ((0, 0), (0, 1), (1, 0)):
                            dst = convq if cv == 0 else convv
                            nn = 512 if cv == 0 else NKV
                            rstep = 34 if cv == 0 else 68
                            cstep = 1 if cv == 0 else 2
                            pcv = ps5.tile([128, nn], F32, tag="ps5")
                            for tap in range(9):
                                dh, dw = tap // 3, tap % 3
                                base = dh * 34 + dw + (
                                    ch * 16 * 34 if cv == 0 else 0)
                                span = 16 * rstep
                                rv = xp[:, base:base + span].rearrange(
                                    "p (r c) -> p r c",
                                    c=rstep)[:, :, 0:32:cstep]
                                nc.tensor.matmul(
                                    pcv, d128_v[:, ct, cv, tap], rv,
                                    start=(tap == 0), stop=(tap == 8))
                            # BN-shift eviction (cast to fp16) on ACT; the
                            # DVE runs the k chains + ctx evicts meanwhile
                            sidx = 0 if cv == 0 else 2
                            nc.scalar.activation(
                                dst[:, ct, ch * nn:(ch + 1) * nn], pcv,
                                mybir.ActivationFunctionType.Identity,
                                bias=shifts_v[:, sidx, ct:ct + 1])
                            yield None
                        # k on the DVE: three independent 3-tap FMA chains
                        # (one per kernel row) + two combines
                        acc = convk[:, ct, :].rearrange("p (r c) -> p r c", c=16)
                        accb = kaccp.tile([128, 2, NKV], F16)
                        accs = [acc,
                                accb[:, 0].rearrange("p (r c) -> p r c", c=16),
                                accb[:, 1].rearrange("p (r c) -> p r c", c=16)]
                        for tap in range(9):
                            dh, dw = tap // 3, tap % 3
                            a = accs[dh]
                            rv = xp[:, dh * 34 + dw:dh * 34 + dw
                                    + 16 * 68].rearrange(
                                "p (r c) -> p r c", c=68)[:, :, 0:32:2]
                            if dw == 0:
                                # chain start; fold the BN shift into chain 0
                                nc.vector.tensor_scalar(
                                    out=a, in0=rv,
                                    scalar1=wcolk_v[:, ct, tap:tap + 1],
                                    scalar2=(shifts_v[:, 1, ct:ct + 1]
                                             if dh == 0 else None),
                                    op0=mybir.AluOpType.mult,
                                    op1=(mybir.AluOpType.add
                                         if dh == 0 else mybir.AluOpType.bypass))
                            else:
                                nc.vector.scalar_tensor_tensor(
                                    out=a, in0=rv,
                                    scalar=wcolk_v[:, ct, tap:tap + 1],
                                    in1=a, op0=mybir.AluOpType.mult,
                                    op1=mybir.AluOpType.add)
                        nc.vector.tensor_tensor(
                            out=accs[1], in0=accs[1], in1=accs[2],
                            op=mybir.AluOpType.add)
                        nc.vector.tensor_tensor(
                            out=acc, in0=acc, in1=accs[1],
                            op=mybir.AluOpType.add)
                        yield None

                return (convq, convk, convv), gen()

            rep_ctx = tc.For_i(0, repeat, 1) if repeat > 1 else None
            if rep_ctx is not None:
                rep_ctx.__enter__()

            tiles_next, gen_next = conv_make(0, prefetch_d128=True)
            for _ in gen_next:
                pass
            # remaining consts + proj weights: issued after conv(0) so the
            # startup-critical transfers win the DMA queues (all are first
            # used later in program order)
            nc.sync.dma_start(biasq_sb, biasq[:, :])
            for kt in range(CT):
                nc.sync.dma_start(wq[kt], w_t[0, kt])
                nc.sync.dma_start(wk[kt], w_t[1, kt])
                nc.sync.dma_start(wv[kt], w_t[2, kt])
            gen_next = iter(())

            for b in range(NB):
                convq, convk, convv = tiles_next

                # ================= projections =================
                qt = big.tile([128, CT, NPIX], F16, tag="qt")
                for et in range(CT):
                    for ch in range(2):
                        pq = ps5.tile([128, 512], F32, tag="ps5")
                        for kt in range(CT):
                            nc.tensor.matmul(
                                pq, wq[kt][:, et * 128:(et + 1) * 128],
                                convq[:, kt, ch * 512:(ch + 1) * 512],
                                start=(kt == 0), stop=(kt == CT - 1))
                        nc.scalar.activation(
                            qt[:, et, ch * 512:(ch + 1) * 512], pq,
                            mybir.ActivationFunctionType.Identity,
                            bias=biasq_sb[:, et:et + 1])

                ktile = big.tile([128, CT, NKV], F16, tag="ktile")
                for et in range(CT):
                    pk = ps5.tile([128, NKV], F32, tag="ps5")
                    for kt in range(CT):
                        nc.tensor.matmul(
                            pk, wk[kt][:, et * 128:(et + 1) * 128],
                            convk[:, kt, :],
                            start=(kt == 0), stop=(kt == CT - 1))
                    nc.vector.tensor_copy(ktile[:, et, :], pk)
                nc.sync.dma_start(kt_out[b], ktile[:, :, :])

                vaug = big.tile([128, 2, HEADS * 128], F16, tag="vaug")
                # softmax-denominator ones-row: col 64 of each head block
                # (cols 65-127 are never read; GpSimd is otherwise idle)
                vaug_ones = vaug.rearrange(
                    "p j (h d) -> p j h d", d=128)[:, :, :, D:D + 1]
                nc.gpsimd.memset(vaug_ones, 1.0)
                for jt in range(2):
                    for ch, (e0, en) in enumerate([(0, 512), (512, 256)]):
                        pv = ps5.tile([128, en], F32, tag="ps5")
                        for kt in range(CT):
                            nc.tensor.matmul(
                                pv, convv[:, kt, jt * 128:(jt + 1) * 128],
                                wv[kt][:, e0:e0 + en],
                                start=(kt == 0), stop=(kt == CT - 1))
                        # scatter heads into the 65-strided V_aug layout
                        h0 = e0 // D
                        nh = en // D
                        dstv = vaug[:, jt, h0 * 128:(h0 + nh) * 128]
                        dstv = dstv.rearrange("p (h d) -> p h d", d=128)[:, :, 0:D]
                        nc.vector.tensor_copy(
                            dstv, pv.rearrange("p (h d) -> p h d", d=D))
                nc.sync.dma_start(
                    v_out[b],
                    vaug.rearrange("p j (h d) -> p j h d", d=128)[:, :, :, 0:D])

                # ================= cls-key scores for all heads =================
                kc = kclsp.tile([128, CT * HEADS], F16)
                nc.sync.dma_start(kc, kcls[b])
                kc_v = kc.rearrange("p (t h) -> p t h", t=CT)
                pcls = ps10.tile([12, NPIX], F32, tag="ps10")
                for ch in range(2):
                    for kt in range(CT):
                        nc.tensor.matmul(
                            pcls[:, ch * 512:(ch + 1) * 512], kc_v[:, kt, :],
                            qt[:, kt, ch * 512:(ch + 1) * 512],
                            start=(kt == 0), stop=(kt == CT - 1))
                ec = eclsp.tile([12, NPIX], F16)
                nc.scalar.activation(ec, pcls, mybir.ActivationFunctionType.Exp,
                                     scale=SM_SCALE)
                nc.sync.dma_start(ecls[b], ec)

                # ================= attention =================
                # software-pipelined one head deep (scores(h+1) before ctx(h))
                # with conv(b+1) PSUM groups interleaved two-per-head to fill
                # PE bubbles while ACT grinds the exps
                def issue_scores(h):
                    g, hh = h // 2, h % 2
                    p0, p1 = 64 * hh, 64 * hh + 64
                    et = [None, None]
                    for jt in range(2):
                        pst = ps10.tile([128, NPIX], F32, tag="ps10")
                        for ch in range(2):
                            nc.tensor.matmul(
                                pst[:, ch * 512:(ch + 1) * 512],
                                ktile[p0:p1, g, jt * 128:(jt + 1) * 128],
                                qt[p0:p1, g, ch * 512:(ch + 1) * 512],
                                start=True, stop=True)
                        ex = expp.tile([128, NPIX], F16)
                        nc.scalar.activation(
                            ex, pst, mybir.ActivationFunctionType.Exp,
                            scale=SM_SCALE)
                        et[jt] = ex
                    return et

                def issue_ctx(h, et):
                    st = stage.tile([D + 1, NPIX], F16)
                    for ch in range(2):
                        pc = psctx.tile([128, 512], F32, tag="psctx")
                        for jt in range(2):
                            nc.tensor.matmul(
                                pc, vaug[:, jt, h * 128:(h + 1) * 128],
                                et[jt][:, ch * 512:(ch + 1) * 512],
                                start=(jt == 0), stop=(jt == 1))
                        nc.vector.tensor_copy(
                            st[:, ch * 512:(ch + 1) * 512], pc[0:D + 1, :])
                    nc.sync.dma_start(ctxu[b, h], st)

                if b + 1 < NB:
                    tiles_next, gen_next = conv_make(b + 1)

                prev = None
                for h in range(HEADS):
                    next(gen_next, None)
                    et = issue_scores(h)
                    if prev is not None:
                        issue_ctx(h - 1, prev)
                    next(gen_next, None)
                    prev = et
                for _ in gen_next:
                    pass
                gen_next = iter(())
                issue_ctx(HEADS - 1, prev)
            if rep_ctx is not None:
                rep_ctx.__exit__(None, None, None)

    nc.finalize()
    return nc


def _host_prep(inputs):
    x = np.ascontiguousarray(inputs["x"], dtype=np.float32)     # [B, 1025, 768]
    B = x.shape[0]
    prep = {}

    cw = {}
    shift = {}
    for i, p in enumerate(["q", "k", "v"]):
        g = np.asarray(inputs[f"bn_g_{p}"], np.float32)
        be = np.asarray(inputs[f"bn_b_{p}"], np.float32)
        m = np.asarray(inputs[f"bn_m_{p}"], np.float32)
        v = np.asarray(inputs[f"bn_v_{p}"], np.float32)
        s = g / np.sqrt(v + EPS)
        cw[p] = np.asarray(inputs[f"conv_w_{p}"], np.float32)[:, 0] * s[:, None, None]
        shift[p] = be - m * s

    # xpad: [B, 6, 128, 1156] = zero-padded 34x34 CHW image, fp16
    hs = x[:, 1:, :].reshape(B, 32, 32, EMBED)
    xp = np.zeros((B, EMBED, 34, 34), np.float16)
    xp[:, :, 1:33, 1:33] = hs.transpose(0, 3, 1, 2)
    prep["xpad"] = np.ascontiguousarray(
        xp.reshape(B, CT, 128, 34 * 34))

    # d128: diag conv weights [128, ct, cv(q,v), tap, 128] fp16
    dd = np.zeros((128, CT, 2, 9, 128), np.float16)
    rng = np.arange(128)
    for ci, p in enumerate(["q", "v"]):
        w9 = cw[p].reshape(EMBED, 9)                  # [c, tap]
        for ct in range(CT):
            dd[rng, ct, ci, :, rng] = w9[ct * 128:(ct + 1) * 128, :].astype(
                np.float16)
    prep["d128"] = np.ascontiguousarray(dd.reshape(128, -1))

    # wcolk: [128, 6*9] per-partition k-conv tap weights (f32)
    w9k = cw["k"].reshape(EMBED, 9)
    prep["wcolk"] = np.ascontiguousarray(
        w9k.reshape(CT, 128, 9).transpose(1, 0, 2).reshape(128, -1))

    # w_t: [3, 6, 128, 768]: W^T split into k-tiles, fp16
    prep["w_t"] = np.ascontiguousarray(np.stack([
        np.asarray(inputs[f"W_{p}"], np.float32).T.reshape(CT, 128, EMBED)
        for p in ["q", "k", "v"]])).astype(np.float16)

    # shifts [128, 3*6] (BN shifts, f32); biasq [128, 6]
    sh = np.stack([shift[p].reshape(CT, 128) for p in ["q", "k", "v"]])  # [3,6,128]
    prep["shifts"] = np.ascontiguousarray(sh.transpose(2, 0, 1).reshape(128, -1))
    prep["biasq"] = np.ascontiguousarray(
        np.asarray(inputs["b_q"], np.float32).reshape(CT, 128).T)

    # host-exact cls projections
    cls = x[:, 0, :]                                               # [B, 768]
    Wq = np.asarray(inputs["W_q"], np.float32)
    Wk = np.asarray(inputs["W_k"], np.float32)
    Wv = np.asarray(inputs["W_v"], np.float32)
    b_k = np.asarray(inputs["b_k"], np.float32)
    b_v = np.asarray(inputs["b_v"], np.float32)
    prep["b_k"] = b_k
    prep["b_v"] = b_v
    prep["q_cls"] = cls @ Wq.T + np.asarray(inputs["b_q"], np.float32)
    k_cls = cls @ Wk.T + b_k
    prep["k_cls"] = k_cls
    prep["v_cls"] = cls @ Wv.T + b_v

    # kcls stuffed block lhsT: [B, 128, 6*12]. The device K rows have no
    # k-bias (cancels in softmax), so remove it from the cls key too.
    k_cls_adj = k_cls - b_k
    kc = np.zeros((B, CT, 128, HEADS), np.float16)
    crange = np.arange(EMBED)
    hofc = crange // D                                             # head of channel
    for b in range(B):
        kc[b, crange // 128, crange % 128, hofc] = k_cls_adj[b].astype(np.float16)
    prep["kcls"] = np.ascontiguousarray(kc.transpose(0, 2, 1, 3).reshape(B, 128, -1))
    return prep


def kernel(**inputs) -> np.ndarray:
    global last_results
    x = np.asarray(inputs["x"], np.float32)
    B = x.shape[0]
    assert B == B_TOTAL, f"kernel hardcoded for B={B_TOTAL}, got {B}"

    prep = _host_prep(inputs)
    nc = _build_program()

    in_maps = []
    for c in range(NCORES):
        sl = slice(c * NB, (c + 1) * NB)
        in_maps.append({
            "xpad": prep["xpad"][sl],
            "d128": prep["d128"],
            "w_t": prep["w_t"],
            "wcolk": prep["wcolk"],
            "shifts": prep["shifts"],
            "biasq": prep["biasq"],
            "kcls": prep["kcls"][sl],
        })

    res = run_bass_kernel_spmd(nc, in_maps, core_ids=list(range(NCORES)))
    last_results = res

    # ---- gather + host combine ----
    ctxu = np.concatenate([r["ctxu"] for r in res.results]).astype(
        np.float32)                                                # [B,12,65,1024]
    ecls = np.concatenate([r["ecls"] for r in res.results]).astype(
        np.float32)                                                # [B,12,1024]
    kto = np.concatenate([r["kt_out"] for r in res.results])       # [B,128,6,256]
    vo = np.concatenate([r["v_out"] for r in res.results])         # [B,128,2,768]

    b_k = prep["b_k"]
    b_v = prep["b_v"]
    # K_conv [B, 256, 768] (device rows lack k-bias; restore for cls row)
    k_conv = kto.astype(np.float32).transpose(0, 3, 2, 1).reshape(B, NKV, EMBED) \
        + b_k
    # V rows [B, 256, 768] (device rows lack v-bias; restore for cls row)
    v5 = vo.astype(np.float32).reshape(B, 128, 2, HEADS, D)        # [B,128,2,12,64]
    v_conv = v5.transpose(0, 2, 1, 3, 4).reshape(B, NKV, EMBED) + b_v

    # vch: bias-less cls V for the device-side merge term
    v_cls = prep["v_cls"]                                          # [B, 768]
    vch = (v_cls - b_v).reshape(B, HEADS, D)
    den = ctxu[:, :, D, :] + ecls                                  # [B,12,1024]
    ctx_pix = (ctxu[:, :, :D, :] + ecls[:, :, None, :] * vch[:, :, :, None]) \
        / den[:, :, None, :]                                       # [B,12,64,1024]
    out = np.empty((B, 1 + NPIX, EMBED), np.float32)
    out[:, 1:, :] = ctx_pix.transpose(0, 3, 1, 2).reshape(B, NPIX, EMBED) \
        + b_v                                                      # restore v-bias

    # cls-query row on host (exact fp32)
    k_all = np.concatenate([prep["k_cls"][:, None, :], k_conv], axis=1)  # [B,257,768]
    v_all = np.concatenate([v_cls[:, None, :], v_conv], axis=1)          # [B,257,768]
    qc = prep["q_cls"].reshape(B, HEADS, D)                              # [B,12,64]
    kh = k_all.reshape(B, 257, HEADS, D)
    vh = v_all.reshape(B, 257, HEADS, D)
    s = np.einsum("bhd,bjhd->bhj", qc, kh) * SM_SCALE
    s -= s.max(axis=2, keepdims=True)
    e = np.exp(s)
    p = e / e.sum(axis=2, keepdims=True)
    ctx0 = np.einsum("bhj,bjhd->bhd", p, vh)                             # [B,12,64]
    out[:, 0, :] = ctx0.reshape(B, EMBED)
    return out


# revision 19
# speedup vs baseline: 2.5002x; 2.5002x over previous
"""CvtSelfAttention TRN2 Bass kernel.

Strategy (8 NeuronCores, batch data-parallel, 4 batch elems per core;
all matmuls fp16 in / fp32 PSUM accumulate):

  1. Depthwise 3x3 convs: q (stride 1) and v (stride 2) as diagonal-weight
     matmuls on the PE (host-precomputed fp16 diag blocks, 9 taps
     accumulate in PSUM, BN shift folded into the ACT eviction); k
     (stride 2) on the Vector engine as three 3-tap per-partition FMA
     chains (scalar_tensor_tensor) + two combines, freeing PE columns.
  2. QKV projections as plain matmuls (contraction over embed):
       QT[e, i] (+b_q at eviction), KT[e, j] (NO k-bias: it cancels in
       softmax; the cls key is adjusted on host), V_aug [j, 12*(64+1)]
       (NO v-bias: added on host; ones-column per head yields softmax
       denominators during the ctx matmul).
  3. Per head: scoresT[j, i] = KT_h^T-slices @ QT_h (contraction 64, head
     pairs on PE row groups 0/64), exp on ACT (scores small, no
     max-subtraction), ctxT_aug[65, i] = V_aug_h^T @ expT.
  4. cls-KEY scores for all 12 heads in one stuffed block-diagonal matmul.

Scheduling: the 24 conv PSUM-groups of batch elem b+1 are interleaved
two-per-head into attention(b) (the PE queue is in-order, so this fills
the exp-latency bubbles); attention itself is software-pipelined one head
deep (scores(h+1) before ctx(h)). Startup-critical DMAs (first c-tile's
diag weights, first image tile) are issued first and split across queues.

Host (numpy, exact fp32): layout prep (padded CHW transpose, BN folding,
fp16 casts), the cls-query attention row (conv K/V rows DMA'd back), the
cls-key merge into ctx, v-bias restore, final [B, L, E] assembly.
"""
import os
import sys
import numpy as np

for _p in ("/opt/trn_rl_repo", "/root/.axon_site/_ro/trn_rl_repo"):
    if os.path.isdir(_p) and _p not in sys.path:
        sys.path.append(_p)

import concourse.bass as bass
import concourse.bacc as bacc
import concourse.tile as tile
from concourse import mybir
from concourse.bass_utils import run_bass_kernel_spmd

EMBED = 768
HEADS = 12
D = 64
EPS = 1e-5
NCORES = 8
B_TOTAL = 32
NB = B_TOTAL // NCORES          # batch elems per core
CT = EMBED // 128               # 6 c-tiles
NPIX = 1024                     # stride-1 conv output pixels (i-dim on device)
NKV = 256                       # stride-2 conv output pixels (j-dim on device)
SM_SCALE = float(EMBED) ** -0.5

F32 = mybir.dt.float32
F16 = mybir.dt.float16

# set by kernel() for test harnesses to inspect
last_results = None


def _build_program(repeat: int = 1):
    nc = bacc.Bacc(None, target_bir_lowering=False, debug=False)

    # ---- DRAM I/O (per core) ----
    xpad = nc.dram_tensor("xpad", [NB, CT, 128, 1156], F16, kind="ExternalInput")
    # host-precomputed diagonal conv weights [128, ct, cv(q,v), tap, 128] fp16
    d128 = nc.dram_tensor("d128", [128, CT * 2 * 9 * 128], F16,
                          kind="ExternalInput")
    w_t = nc.dram_tensor("w_t", [3, CT, 128, EMBED], F16, kind="ExternalInput")
    wcolk = nc.dram_tensor("wcolk", [128, CT * 9], F32, kind="ExternalInput")
    shifts = nc.dram_tensor("shifts", [128, 3 * CT], F32, kind="ExternalInput")
    biasq = nc.dram_tensor("biasq", [128, CT], F32, kind="ExternalInput")
    kcls = nc.dram_tensor("kcls", [NB, 128, CT * HEADS], F16, kind="ExternalInput")

    ctxu = nc.dram_tensor("ctxu", [NB, HEADS, D + 1, NPIX], F16,
                          kind="ExternalOutput")
    ecls = nc.dram_tensor("ecls", [NB, HEADS, NPIX], F16, kind="ExternalOutput")
    kt_out = nc.dram_tensor("kt_out", [NB, 128, CT, NKV], F16,
                            kind="ExternalOutput")
    v_out = nc.dram_tensor("v_out", [NB, 128, 2, HEADS * D], F16,
                           kind="ExternalOutput")

    with tile.TileContext(nc) as tc:
        import contextlib
        with contextlib.ExitStack() as ctx:
            consts = ctx.enter_context(tc.tile_pool(name="consts", bufs=1))
            kclsp = ctx.enter_context(tc.tile_pool(name="kclsp", bufs=2))
            xpool = ctx.enter_context(tc.tile_pool(name="xpool", bufs=3))
            kaccp = ctx.enter_context(tc.tile_pool(name="kaccp", bufs=2))
            big = ctx.enter_context(tc.tile_pool(name="big", bufs=2))
            expp = ctx.enter_context(tc.tile_pool(name="expp", bufs=6))
            stage = ctx.enter_context(tc.tile_pool(name="stage", bufs=4))
            eclsp = ctx.enter_context(tc.tile_pool(name="eclsp", bufs=2))
            ps5 = ctx.enter_context(tc.tile_pool(name="ps5", bufs=2, space="PSUM"))
            ps10 = ctx.enter_context(tc.tile_pool(name="ps10", bufs=2, space="PSUM"))
            psctx = ctx.enter_context(tc.tile_pool(name="psctx", bufs=2, space="PSUM"))

            # ---- constants ----
            # startup-critical consts first: the first c-tile's diag weights
            # (split across queues), BN shifts, k tap scalars. Everything else
            # is DMA'd after conv(b0) is emitted so the xpad transfers win
            # the DMA queues.
            d128_sb = consts.tile([128, CT * 2 * 9 * 128], F16)
            d128_v = d128_sb.rearrange("p (t c k s) -> p t c k s", t=CT, c=2,
                                       k=9)
            d128_dv = d128[:, :].rearrange("p (t c k s) -> p t c k s", t=CT,
                                           c=2, k=9)
            for kk in range(3):
                nc.sync.dma_start(d128_v[:, 0, 0, 3 * kk:3 * kk + 3],
                                  d128_dv[:, 0, 0, 3 * kk:3 * kk + 3])
            shifts_sb = consts.tile([128, 3 * CT], F32)
            shifts_v = shifts_sb.rearrange("p (c t) -> p c t", c=3)
            wcolk_sb = consts.tile([128, CT * 9], F32)
            wcolk_v = wcolk_sb.rearrange("p (t k) -> p t k", t=CT)
            biasq_sb = consts.tile([128, CT], F32)
            wq = [consts.tile([128, EMBED], F16, name=f"wq{i}") for i in range(CT)]
            wk = [consts.tile([128, EMBED], F16, name=f"wk{i}") for i in range(CT)]
            wv = [consts.tile([128, EMBED], F16, name=f"wv{i}") for i in range(CT)]

            def conv_make(b, prefetch_d128=False):
                """Allocate b's conv tiles; return (tiles, generator) where the
                generator emits one PSUM group (9 diag matmuls + eviction) per
                next() so conv(b) can interleave into attention(b-1)."""
                convq = big.tile([128, CT, NPIX], F16, tag="convq")
                convk = big.tile([128, CT, NKV], F16, tag="convk")
                convv = big.tile([128, CT, NKV], F16, tag="convv")

                def gen():
                    for ct in range(CT):
                        if prefetch_d128 and ct + 1 < CT:
                            # b0 only: pull in the next c-tile's diag weights
                            # (must precede their first use in program order)
                            nc.sync.dma_start(d128_v[:, ct + 1],
                                              d128_dv[:, ct + 1])
                        # 1164 = 1156 + slack so tap-shifted row views stay
                        # in-bounds (their trailing cols are never read)
                        xp = xpool.tile([128, 1164], F16)
                        if prefetch_d128:
                            nc.sync.dma_start(xp[:, 0:616], xpad[b, ct][:, 0:616])
                            nc.sync.dma_start(xp[:, 616:1156],
                                              xpad[b, ct][:, 616:1156])
                            if ct == 0:
                                nc.sync.dma_start(shifts_sb, shifts[:, :])
                                nc.sync.dma_start(wcolk_sb, wcolk[:, :])
                                nc.sync.dma_start(d128_v[:, 0, 1],
                                                  d128_dv[:, 0, 1])
                        else:
                            nc.sync.dma_start(xp[:, 0:1156], xpad[b, ct])
                        # q (2 chunks) and v on the PE as fp16 diag matmuls
                        for cv, ch in ((0, 0), (0, 1), (1, 0)):
                            dst = convq if cv == 0 else convv
                            nn = 512 if cv == 0 else NKV
                            rstep = 34 if cv == 0 else 68
                            cstep = 1 if cv == 0 else 2
                            pcv = ps5.tile([128, nn], F32, tag="ps5")
                            for tap in range(9):
                                dh, dw = tap // 3, tap % 3
                                base = dh * 34 + dw + (
                                    ch * 16 * 34 if cv == 0 else 0)
                                span = 16 * rstep
                                rv = xp[:, base:base + span].rearrange(
                                    "p (r c) -> p r c",
                                    c=rstep)[:, :, 0:32:cstep]
                                nc.tensor.matmul(
                                    pcv, d128_v[:, ct, cv, tap], rv,
                                    start=(tap == 0), stop=(tap == 8))
                            # BN-shift eviction (cast to fp16) on ACT; the
                            # DVE runs the k chains + ctx evicts meanwhile
                            sidx = 0 if cv == 0 else 2
                            nc.scalar.activation(
                                dst[:, ct, ch * nn:(ch + 1) * nn], pcv,
                                mybir.ActivationFunctionType.Identity,
                                bias=shifts_v[:, sidx, ct:ct + 1])
                            yield None
                        # k on the DVE: three independent 3-tap FMA chains
                        # (one per kernel row) + two combines
                        acc = convk[:, ct, :].rearrange("p (r c) -> p r c", c=16)
                        accb = kaccp.tile([128, 2, NKV], F16)
                        accs = [acc,
                                accb[:, 0].rearrange("p (r c) -> p r c", c=16),
                                accb[:, 1].rearrange("p (r c) -> p r c", c=16)]
                        for tap in range(9):
                            dh, dw = tap // 3, tap % 3
                            a = accs[dh]
                            rv = xp[:, dh * 34 + dw:dh * 34 + dw
                                    + 16 * 68].rearrange(
                                "p (r c) -> p r c", c=68)[:, :, 0:32:2]
                            if dw == 0:
                                # chain start; fold the BN shift into chain 0
                                nc.vector.tensor_scalar(
                                    out=a, in0=rv,
                                    scalar1=wcolk_v[:, ct, tap:tap + 1],
                                    scalar2=(shifts_v[:, 1, ct:ct + 1]
                                             if dh == 0 else None),
                                    op0=mybir.AluOpType.mult,
                                    op1=(mybir.AluOpType.add
                                         if dh == 0 else mybir.AluOpType.bypass))
                            else:
                                nc.vector.scalar_tensor_tensor(
                                    out=a, in0=rv,
                                    scalar=wcolk_v[:, ct, tap:tap + 1],
                                    in1=a, op0=mybir.AluOpType.mult,
                                    op1=mybir.AluOpType.add)
                        nc.vector.tensor_tensor(
                            out=accs[1], in0=accs[1], in1=accs[2],
                            op=mybir.AluOpType.add)
                        nc.vector.tensor_tensor(
                            out=acc, in0=acc, in1=accs[1],
                            op=mybir.AluOpType.add)
                        yield None

                return (convq, convk, convv), gen()

            rep_ctx = tc.For_i(0, repeat, 1) if repeat > 1 else None
            if rep_ctx is not None:
                rep_ctx.__enter__()

            tiles_next, gen_next = conv_make(0, prefetch_d128=True)
            for _ in gen_next:
                pass
            # remaining consts + proj weights: issued after conv(0) so the
            # startup-critical transfers win the DMA queues (all are first
            # used later in program order)
            nc.sync.dma_start(biasq_sb, biasq[:, :])
            for kt in range(CT):
                nc.sync.dma_start(wq[kt], w_t[0, kt])
                nc.sync.dma_start(wk[kt], w_t[1, kt])
                nc.sync.dma_start(wv[kt], w_t[2, kt])
            gen_next = iter(())

            for b in range(NB):
                convq, convk, convv = tiles_next

                # ================= projections =================
                qt = big.tile([128, CT, NPIX], F16, tag="qt")
                for et in range(CT):
                    for ch in range(2):
                        pq = ps5.tile([128, 512], F32, tag="ps5")
                        for kt in range(CT):
                            nc.tensor.matmul(
                                pq, wq[kt][:, et * 128:(et + 1) * 128],
                                convq[:, kt, ch * 512:(ch + 1) * 512],
                                start=(kt == 0), stop=(kt == CT - 1))
                        nc.scalar.activation(
                            qt[:, et, ch * 512:(ch + 1) * 512], pq,
                            mybir.ActivationFunctionType.Identity,
                            bias=biasq_sb[:, et:et + 1])

                ktile = big.tile([128, CT, NKV], F16, tag="ktile")
                for et in range(CT):
                    pk = ps5.tile([128, NKV], F32, tag="ps5")
                    for kt in range(CT):
                        nc.tensor.matmul(
                            pk, wk[kt][:, et * 128:(et + 1) * 128],
                            convk[:, kt, :],
                            start=(kt == 0), stop=(kt == CT - 1))
                    nc.vector.tensor_copy(ktile[:, et, :], pk)
                nc.sync.dma_start(kt_out[b], ktile[:, :, :])

                vaug = big.tile([128, 2, HEADS * 128], F16, tag="vaug")
                # softmax-denominator ones-row: col 64 of each head block
                # (cols 65-127 are never read; GpSimd is otherwise idle)
                vaug_ones = vaug.rearrange(
                    "p j (h d) -> p j h d", d=128)[:, :, :, D:D + 1]
                nc.gpsimd.memset(vaug_ones, 1.0)
                for jt in range(2):
                    for ch, (e0, en) in enumerate([(0, 512), (512, 256)]):
                        pv = ps5.tile([128, en], F32, tag="ps5")
                        for kt in range(CT):
                            nc.tensor.matmul(
                                pv, convv[:, kt, jt * 128:(jt + 1) * 128],
                                wv[kt][:, e0:e0 + en],
                                start=(kt == 0), stop=(kt == CT - 1))
                        # scatter heads into the 65-strided V_aug layout
                        h0 = e0 // D
                        nh = en // D
                        dstv = vaug[:, jt, h0 * 128:(h0 + nh) * 128]
                        dstv = dstv.rearrange("p (h d) -> p h d", d=128)[:, :, 0:D]
                        nc.vector.tensor_copy(
                            dstv, pv.rearrange("p (h d) -> p h d", d=D))
                nc.sync.dma_start(
                    v_out[b],
                    vaug.rearrange("p j (h d) -> p j h d", d=128)[:, :, :, 0:D])

                # ================= cls-key scores for all heads =================
                kc = kclsp.tile([128, CT * HEADS], F16)
                nc.sync.dma_start(kc, kcls[b])
                kc_v = kc.rearrange("p (t h) -> p t h", t=CT)
                pcls = ps10.tile([12, NPIX], F32, tag="ps10")
                for ch in range(2):
                    for kt in range(CT):
                        nc.tensor.matmul(
                            pcls[:, ch * 512:(ch + 1) * 512], kc_v[:, kt, :],
                            qt[:, kt, ch * 512:(ch + 1) * 512],
                            start=(kt == 0), stop=(kt == CT - 1))
                ec = eclsp.tile([12, NPIX], F16)
                nc.scalar.activation(ec, pcls, mybir.ActivationFunctionType.Exp,
                                     scale=SM_SCALE)
                nc.sync.dma_start(ecls[b], ec)

                # ================= attention =================
                # software-pipelined one head deep (scores(h+1) before ctx(h))
                # with conv(b+1) PSUM groups interleaved two-per-head to fill
                # PE bubbles while ACT grinds the exps
                def issue_scores(h):
                    g, hh = h // 2, h % 2
                    p0, p1 = 64 * hh, 64 * hh + 64
                    et = [None, None]
                    for jt in range(2):
                        pst = ps10.tile([128, NPIX], F32, tag="ps10")
                        for ch in range(2):
                            nc.tensor.matmul(
                                pst[:, ch * 512:(ch + 1) * 512],
                                ktile[p0:p1, g, jt * 128:(jt + 1) * 128],
                                qt[p0:p1, g, ch * 512:(ch + 1) * 512],
                                start=True, stop=True)
                        ex = expp.tile([128, NPIX], F16)
                        nc.scalar.activation(
                            ex, pst, mybir.ActivationFunctionType.Exp,
                            scale=SM_SCALE)
                        et[jt] = ex
                    return et

                def issue_ctx(h, et):
                    st = stage.tile([D + 1, NPIX], F16)
                    for ch in range(2):
                        pc = psctx.tile([128, 512], F32, tag="psctx")
                        for jt in range(2):
                            nc.tensor.matmul(
                                pc, vaug[:, jt, h * 128:(h + 1) * 128],
                                et[jt][:, ch * 512:(ch + 1) * 512],
                                start=(jt == 0), stop=(jt == 1))
                        nc.vector.tensor_copy(
                            st[:, ch * 512:(ch + 1) * 512], pc[0:D + 1, :])
                    nc.sync.dma_start(ctxu[b, h], st)

                if b + 1 < NB:
                    tiles_next, gen_next = conv_make(b + 1)

                prev = None
                for h in range(HEADS):
                    next(gen_next, None)
                    et = issue_scores(h)
                    if prev is not None:
                        issue_ctx(h - 1, prev)
                    next(gen_next, None)
                    prev = et
                for _ in gen_next:
                    pass
                gen_next = iter(())
                issue_ctx(HEADS - 1, prev)
            if rep_ctx is not None:
                rep_ctx.__exit__(None, None, None)

    nc.finalize()
    return nc


def _host_prep(inputs):
    x = np.ascontiguousarray(inputs["x"], dtype=np.float32)     # [B, 1025, 768]
    B = x.shape[0]
    prep = {}

    cw = {}
    shift = {}
    for i, p in enumerate(["q", "k", "v"]):
        g = np.asarray(inputs[f"bn_g_{p}"], np.float32)
        be = np.asarray(inputs[f"bn_b_{p}"], np.float32)
        m = np.asarray(inputs[f"bn_m_{p}"], np.float32)
        v = np.asarray(inputs[f"bn_v_{p}"], np.float32)
        s = g / np.sqrt(v + EPS)
        cw[p] = np.asarray(inputs[f"conv_w_{p}"], np.float32)[:, 0] * s[:, None, None]
        shift[p] = be - m * s

    # xpad: [B, 6, 128, 1156] = zero-padded 34x34 CHW image, fp16
    hs = x[:, 1:, :].reshape(B, 32, 32, EMBED)
    xp = np.zeros((B, EMBED, 34, 34), np.float16)
    xp[:, :, 1:33, 1:33] = hs.transpose(0, 3, 1, 2)
    prep["xpad"] = np.ascontiguousarray(
        xp.reshape(B, CT, 128, 34 * 34))

    # d128: diag conv weights [128, ct, cv(q,v), tap, 128] fp16
    dd = np.zeros((128, CT, 2, 9, 128), np.float16)
    rng = np.arange(128)
    for ci, p in enumerate(["q", "v"]):
        w9 = cw[p].reshape(EMBED, 9)                  # [c, tap]
        for ct in range(CT):
            dd[rng, ct, ci, :, rng] = w9[ct * 128:(ct + 1) * 128, :].astype(
                np.float16)
    prep["d128"] = np.ascontiguousarray(dd.reshape(128, -1))

    # wcolk: [128, 6*9] per-partition k-conv tap weights (f32)
    w9k = cw["k"].reshape(EMBED, 9)
    prep["wcolk"] = np.ascontiguousarray(
        w9k.reshape(CT, 128, 9).transpose(1, 0, 2).reshape(128, -1))

    # w_t: [3, 6, 128, 768]: W^T split into k-tiles, fp16
    prep["w_t"] = np.ascontiguousarray(np.stack([
        np.asarray(inputs[f"W_{p}"], np.float32).T.reshape(CT, 128, EMBED)
        for p in ["q", "k", "v"]])).astype(np.float16)

    # shifts [128, 3*6] (BN shifts, f32); biasq [128, 6]
    sh = np.stack([shift[p].reshape(CT, 128) for p in ["q", "k", "v"]])  # [3,6,128]
    prep["shifts"] = np.ascontiguousarray(sh.transpose(2, 0, 1).reshape(128, -1))
    prep["biasq"] = np.ascontiguousarray(
        np.asarray(inputs["b_q"], np.float32).reshape(CT, 128).T)

    # host-exact cls projections
    cls = x[:, 0, :]                                               # [B, 768]
    Wq = np.asarray(inputs["W_q"], np.float32)
    Wk = np.asarray(inputs["W_k"], np.float32)
    Wv = np.asarray(inputs["W_v"], np.float32)
    b_k = np.asarray(inputs["b_k"], np.float32)
    b_v = np.asarray(inputs["b_v"], np.float32)
    prep["b_k"] = b_k
    prep["b_v"] = b_v
    prep["q_cls"] = cls @ Wq.T + np.asarray(inputs["b_q"], np.float32)
    k_cls = cls @ Wk.T + b_k
    prep["k_cls"] = k_cls
    prep["v_cls"] = cls @ Wv.T + b_v

    # kcls stuffed block lhsT: [B, 128, 6*12]. The device K rows have no
    # k-bias (cancels in softmax), so remove it from the cls key too.
    k_cls_adj = k_cls - b_k
    kc = np.zeros((B, CT, 128, HEADS), np.float16)
    crange = np.arange(EMBED)
    hofc = crange // D                                             # head of channel
    for b in range(B):
        kc[b, crange // 128, crange % 128, hofc] = k_cls_adj[b].astype(np.float16)
    prep["kcls"] = np.ascontiguousarray(kc.transpose(0, 2, 1, 3).reshape(B, 128, -1))
    return prep


def kernel(**inputs) -> np.ndarray:
    global last_results
    x = np.asarray(inputs["x"], np.float32)
    B = x.shape[0]
    assert B == B_TOTAL, f"kernel hardcoded for B={B_TOTAL}, got {B}"

    prep = _host_prep(inputs)
    nc = _build_program()

    in_maps = []
    for c in range(NCORES):
        sl = slice(c * NB, (c + 1) * NB)
        in_maps.append({
            "xpad": prep["xpad"][sl],
            "d128": prep["d128"],
            "w_t": prep["w_t"],
            "wcolk": prep["wcolk"],
            "shifts": prep["shifts"],
            "biasq": prep["biasq"],
            "kcls": prep["kcls"][sl],
        })

    res = run_bass_kernel_spmd(nc, in_maps, core_ids=list(range(NCORES)))
    last_results = res

    # ---- gather + host combine ----
    ctxu = np.concatenate([r["ctxu"] for r in res.results]).astype(
        np.float32)                                                # [B,12,65,1024]
    ecls = np.concatenate([r["ecls"] for r in res.results]).astype(
        np.float32)                                                # [B,12,1024]
    kto = np.concatenate([r["kt_out"] for r in res.results])       # [B,128,6,256]
    vo = np.concatenate([r["v_out"] for r in res.results])         # [B,128,2,768]

    b_k = prep["b_k"]
    b_v = prep["b_v"]
    # K_conv [B, 256, 768] (device rows lack k-bias; restore for cls row)
    k_conv = kto.astype(np.float32).transpose(0, 3, 2, 1).reshape(B, NKV, EMBED) \
        + b_k
    # V rows [B, 256, 768] (device rows lack v-bias; restore for cls row)
    v5 = vo.astype(np.float32).reshape(B, 128, 2, HEADS, D)        # [B,128,2,12,64]
    v_conv = v5.transpose(0, 2, 1, 3, 4).reshape(B, NKV, EMBED) + b_v

    # vch: bias-less cls V for the device-side merge term
    v_cls = prep["v_cls"]                                          # [B, 768]
    vch = (v_cls - b_v).reshape(B, HEADS, D)
    den = ctxu[:, :, D, :] + ecls                                  # [B,12,1024]
    ctx_pix = (ctxu[:, :, :D, :] + ecls[:, :, None, :] * vch[:, :, :, None]) \
        / den[:, :, None, :]                                       # [B,12,64,1024]
    out = np.empty((B, 1 + NPIX, EMBED), np.float32)
    out[:, 1:, :] = ctx_pix.transpose(0, 3, 1, 2).reshape(B, NPIX, EMBED) \
        + b_v                                                      # restore v-bias

    # cls-query row on host (exact fp32)
    k_all = np.concatenate([prep["k_cls"][:, None, :], k_conv], axis=1)  # [B,257,768]
    v_all = np.concatenate([v_cls[:, None, :], v_conv], axis=1)          # [B,257,768]
    qc = prep["q_cls"].reshape(B, HEADS, D)                              # [B,12,64]
    kh = k_all.reshape(B, 257, HEADS, D)
    vh = v_all.reshape(B, 257, HEADS, D)
    s = np.einsum("bhd,bjhd->bhj", qc, kh) * SM_SCALE
    s -= s.max(axis=2, keepdims=True)
    e = np.exp(s)
    p = e / e.sum(axis=2, keepdims=True)
    ctx0 = np.einsum("bhj,bjhd->bhd", p, vh)                             # [B,12,64]
    out[:, 0, :] = ctx0.reshape(B, EMBED)
    return out


# revision 20
# speedup vs baseline: 2.5115x; 1.0045x over previous
"""CvtSelfAttention TRN2 Bass kernel.

Strategy (8 NeuronCores, batch data-parallel, 4 batch elems per core;
all matmuls fp16 in / fp32 PSUM accumulate):

  1. Depthwise 3x3 convs: q (stride 1) and v (stride 2) as diagonal-weight
     matmuls on the PE (host-precomputed fp16 diag blocks, 9 taps
     accumulate in PSUM, BN shift folded into the ACT eviction); k
     (stride 2) on the Vector engine as three 3-tap per-partition FMA
     chains (scalar_tensor_tensor) + two combines, freeing PE columns.
  2. QKV projections as plain matmuls (contraction over embed):
       QT[e, i] (+b_q at eviction), KT[e, j] (NO k-bias: it cancels in
       softmax; the cls key is adjusted on host), V_aug [j, 12*(64+1)]
       (NO v-bias: added on host; ones-column per head yields softmax
       denominators during the ctx matmul).
  3. Per head: scoresT[j, i] = KT_h^T-slices @ QT_h (contraction 64, head
     pairs on PE row groups 0/64), exp on ACT (scores small, no
     max-subtraction), ctxT_aug[65, i] = V_aug_h^T @ expT.
  4. cls-KEY scores for all 12 heads in one stuffed block-diagonal matmul.

Scheduling: the 24 conv PSUM-groups of batch elem b+1 are interleaved
two-per-head into attention(b) (the PE queue is in-order, so this fills
the exp-latency bubbles); attention itself is software-pipelined one head
deep (scores(h+1) before ctx(h)). Startup-critical DMAs (first c-tile's
diag weights, first image tile) are issued first and split across queues.

Host (numpy, exact fp32): layout prep (padded CHW transpose, BN folding,
fp16 casts), the cls-query attention row (conv K/V rows DMA'd back), the
cls-key merge into ctx, v-bias restore, final [B, L, E] assembly.
"""
import os
import sys
import numpy as np

for _p in ("/opt/trn_rl_repo", "/root/.axon_site/_ro/trn_rl_repo"):
    if os.path.isdir(_p) and _p not in sys.path:
        sys.path.append(_p)

import concourse.bass as bass
import concourse.bacc as bacc
import concourse.tile as tile
from concourse import mybir
from concourse.bass_utils import run_bass_kernel_spmd

EMBED = 768
HEADS = 12
D = 64
EPS = 1e-5
NCORES = 8
B_TOTAL = 32
NB = B_TOTAL // NCORES          # batch elems per core
CT = EMBED // 128               # 6 c-tiles
NPIX = 1024                     # stride-1 conv output pixels (i-dim on device)
NKV = 256                       # stride-2 conv output pixels (j-dim on device)
SM_SCALE = float(EMBED) ** -0.5

F32 = mybir.dt.float32
F16 = mybir.dt.float16

# set by kernel() for test harnesses to inspect
last_results = None


def _build_program(repeat: int = 1):
    nc = bacc.Bacc(None, target_bir_lowering=False, debug=False)

    # ---- DRAM I/O (per core) ----
    xpad = nc.dram_tensor("xpad", [NB, CT, 128, 1156], F16, kind="ExternalInput")
    # host-precomputed diagonal conv weights [128, ct, cv(q,v), tap, 128] fp16
    d128 = nc.dram_tensor("d128", [128, CT * 2 * 9 * 128], F16,
                          kind="ExternalInput")
    w_t = nc.dram_tensor("w_t", [3, CT, 128, EMBED], F16, kind="ExternalInput")
    wcolk = nc.dram_tensor("wcolk", [128, CT * 9], F32, kind="ExternalInput")
    shifts = nc.dram_tensor("shifts", [128, 3 * CT], F32, kind="ExternalInput")
    biasq = nc.dram_tensor("biasq", [128, CT], F32, kind="ExternalInput")
    kcls = nc.dram_tensor("kcls", [NB, 128, CT * HEADS], F16, kind="ExternalInput")

    ctxu = nc.dram_tensor("ctxu", [NB, HEADS, D + 1, NPIX], F16,
                          kind="ExternalOutput")
    ecls = nc.dram_tensor("ecls", [NB, HEADS, NPIX], F16, kind="ExternalOutput")
    kt_out = nc.dram_tensor("kt_out", [NB, 128, CT, NKV], F16,
                            kind="ExternalOutput")
    v_out = nc.dram_tensor("v_out", [NB, 128, 2, HEADS * D], F16,
                           kind="ExternalOutput")

    with tile.TileContext(nc) as tc:
        import contextlib
        with contextlib.ExitStack() as ctx:
            consts = ctx.enter_context(tc.tile_pool(name="consts", bufs=1))
            kclsp = ctx.enter_context(tc.tile_pool(name="kclsp", bufs=2))
            xpool = ctx.enter_context(tc.tile_pool(name="xpool", bufs=4))
            kaccp = ctx.enter_context(tc.tile_pool(name="kaccp", bufs=3))
            big = ctx.enter_context(tc.tile_pool(name="big", bufs=2))
            expp = ctx.enter_context(tc.tile_pool(name="expp", bufs=8))
            stage = ctx.enter_context(tc.tile_pool(name="stage", bufs=6))
            eclsp = ctx.enter_context(tc.tile_pool(name="eclsp", bufs=2))
            ps5 = ctx.enter_context(tc.tile_pool(name="ps5", bufs=2, space="PSUM"))
            ps10 = ctx.enter_context(tc.tile_pool(name="ps10", bufs=2, space="PSUM"))
            psctx = ctx.enter_context(tc.tile_pool(name="psctx", bufs=2, space="PSUM"))

            # ---- constants ----
            # startup-critical consts first: the first c-tile's diag weights
            # (split across queues), BN shifts, k tap scalars. Everything else
            # is DMA'd after conv(b0) is emitted so the xpad transfers win
            # the DMA queues.
            d128_sb = consts.tile([128, CT * 2 * 9 * 128], F16)
            d128_v = d128_sb.rearrange("p (t c k s) -> p t c k s", t=CT, c=2,
                                       k=9)
            d128_dv = d128[:, :].rearrange("p (t c k s) -> p t c k s", t=CT,
                                           c=2, k=9)
            for kk in range(3):
                nc.sync.dma_start(d128_v[:, 0, 0, 3 * kk:3 * kk + 3],
                                  d128_dv[:, 0, 0, 3 * kk:3 * kk + 3])
            shifts_sb = consts.tile([128, 3 * CT], F32)
            shifts_v = shifts_sb.rearrange("p (c t) -> p c t", c=3)
            wcolk_sb = consts.tile([128, CT * 9], F32)
            wcolk_v = wcolk_sb.rearrange("p (t k) -> p t k", t=CT)
            biasq_sb = consts.tile([128, CT], F32)
            wq = [consts.tile([128, EMBED], F16, name=f"wq{i}") for i in range(CT)]
            wk = [consts.tile([128, EMBED], F16, name=f"wk{i}") for i in range(CT)]
            wv = [consts.tile([128, EMBED], F16, name=f"wv{i}") for i in range(CT)]

            def conv_make(b, prefetch_d128=False):
                """Allocate b's conv tiles; return (tiles, generator) where the
                generator emits one PSUM group (9 diag matmuls + eviction) per
                next() so conv(b) can interleave into attention(b-1)."""
                convq = big.tile([128, CT, NPIX], F16, tag="convq")
                convk = big.tile([128, CT, NKV], F16, tag="convk")
                convv = big.tile([128, CT, NKV], F16, tag="convv")

                def gen():
                    for ct in range(CT):
                        if prefetch_d128 and ct + 1 < CT:
                            # b0 only: pull in the next c-tile's diag weights
                            # (must precede their first use in program order)
                            nc.sync.dma_start(d128_v[:, ct + 1],
                                              d128_dv[:, ct + 1])
                        # 1164 = 1156 + slack so tap-shifted row views stay
                        # in-bounds (their trailing cols are never read)
                        xp = xpool.tile([128, 1164], F16)
                        if prefetch_d128:
                            nc.sync.dma_start(xp[:, 0:616], xpad[b, ct][:, 0:616])
                            nc.sync.dma_start(xp[:, 616:1156],
                                              xpad[b, ct][:, 616:1156])
                            if ct == 0:
                                nc.sync.dma_start(shifts_sb, shifts[:, :])
                                nc.sync.dma_start(wcolk_sb, wcolk[:, :])
                                nc.sync.dma_start(d128_v[:, 0, 1],
                                                  d128_dv[:, 0, 1])
                        else:
                            nc.sync.dma_start(xp[:, 0:1156], xpad[b, ct])
                        # q (2 chunks) and v on the PE as fp16 diag matmuls
                        for cv, ch in ((0, 0), (0, 1), (1, 0)):
                            dst = convq if cv == 0 else convv
                            nn = 512 if cv == 0 else NKV
                            rstep = 34 if cv == 0 else 68
                            cstep = 1 if cv == 0 else 2
                            pcv = ps5.tile([128, nn], F32, tag="ps5")
                            for tap in range(9):
                                dh, dw = tap // 3, tap % 3
                                base = dh * 34 + dw + (
                                    ch * 16 * 34 if cv == 0 else 0)
                                span = 16 * rstep
                                rv = xp[:, base:base + span].rearrange(
                                    "p (r c) -> p r c",
                                    c=rstep)[:, :, 0:32:cstep]
                                nc.tensor.matmul(
                                    pcv, d128_v[:, ct, cv, tap], rv,
                                    start=(tap == 0), stop=(tap == 8))
                            # BN-shift eviction (cast to fp16) on ACT; the
                            # DVE runs the k chains + ctx evicts meanwhile
                            sidx = 0 if cv == 0 else 2
                            nc.scalar.activation(
                                dst[:, ct, ch * nn:(ch + 1) * nn], pcv,
                                mybir.ActivationFunctionType.Identity,
                                bias=shifts_v[:, sidx, ct:ct + 1])
                            yield None
                        # k on the DVE: three independent 3-tap FMA chains
                        # (one per kernel row) + two combines
                        acc = convk[:, ct, :].rearrange("p (r c) -> p r c", c=16)
                        accb = kaccp.tile([128, 2, NKV], F16)
                        accs = [acc,
                                accb[:, 0].rearrange("p (r c) -> p r c", c=16),
                                accb[:, 1].rearrange("p (r c) -> p r c", c=16)]
                        for tap in range(9):
                            dh, dw = tap // 3, tap % 3
                            a = accs[dh]
                            rv = xp[:, dh * 34 + dw:dh * 34 + dw
                                    + 16 * 68].rearrange(
                                "p (r c) -> p r c", c=68)[:, :, 0:32:2]
                            if dw == 0:
                                # chain start; fold the BN shift into chain 0
                                nc.vector.tensor_scalar(
                                    out=a, in0=rv,
                                    scalar1=wcolk_v[:, ct, tap:tap + 1],
                                    scalar2=(shifts_v[:, 1, ct:ct + 1]
                                             if dh == 0 else None),
                                    op0=mybir.AluOpType.mult,
                                    op1=(mybir.AluOpType.add
                                         if dh == 0 else mybir.AluOpType.bypass))
                            else:
                                nc.vector.scalar_tensor_tensor(
                                    out=a, in0=rv,
                                    scalar=wcolk_v[:, ct, tap:tap + 1],
                                    in1=a, op0=mybir.AluOpType.mult,
                                    op1=mybir.AluOpType.add)
                        nc.vector.tensor_tensor(
                            out=accs[1], in0=accs[1], in1=accs[2],
                            op=mybir.AluOpType.add)
                        nc.vector.tensor_tensor(
                            out=acc, in0=acc, in1=accs[1],
                            op=mybir.AluOpType.add)
                        yield None

                return (convq, convk, convv), gen()

            rep_ctx = tc.For_i(0, repeat, 1) if repeat > 1 else None
            if rep_ctx is not None:
                rep_ctx.__enter__()

            tiles_next, gen_next = conv_make(0, prefetch_d128=True)
            for _ in gen_next:
                pass
            # remaining consts + proj weights: issued after conv(0) so the
            # startup-critical transfers win the DMA queues (all are first
            # used later in program order)
            nc.sync.dma_start(biasq_sb, biasq[:, :])
            for kt in range(CT):
                nc.sync.dma_start(wq[kt], w_t[0, kt])
                nc.sync.dma_start(wk[kt], w_t[1, kt])
                nc.sync.dma_start(wv[kt], w_t[2, kt])
            gen_next = iter(())

            for b in range(NB):
                convq, convk, convv = tiles_next

                # ================= projections =================
                qt = big.tile([128, CT, NPIX], F16, tag="qt")
                for et in range(CT):
                    for ch in range(2):
                        pq = ps5.tile([128, 512], F32, tag="ps5")
                        for kt in range(CT):
                            nc.tensor.matmul(
                                pq, wq[kt][:, et * 128:(et + 1) * 128],
                                convq[:, kt, ch * 512:(ch + 1) * 512],
                                start=(kt == 0), stop=(kt == CT - 1))
                        nc.scalar.activation(
                            qt[:, et, ch * 512:(ch + 1) * 512], pq,
                            mybir.ActivationFunctionType.Identity,
                            bias=biasq_sb[:, et:et + 1])

                ktile = big.tile([128, CT, NKV], F16, tag="ktile")
                for et in range(CT):
                    pk = ps5.tile([128, NKV], F32, tag="ps5")
                    for kt in range(CT):
                        nc.tensor.matmul(
                            pk, wk[kt][:, et * 128:(et + 1) * 128],
                            convk[:, kt, :],
                            start=(kt == 0), stop=(kt == CT - 1))
                    nc.vector.tensor_copy(ktile[:, et, :], pk)
                nc.sync.dma_start(kt_out[b], ktile[:, :, :])

                vaug = big.tile([128, 2, HEADS * 128], F16, tag="vaug")
                # softmax-denominator ones-row: col 64 of each head block
                # (cols 65-127 are never read; GpSimd is otherwise idle)
                vaug_ones = vaug.rearrange(
                    "p j (h d) -> p j h d", d=128)[:, :, :, D:D + 1]
                nc.gpsimd.memset(vaug_ones, 1.0)
                for jt in range(2):
                    for ch, (e0, en) in enumerate([(0, 512), (512, 256)]):
                        pv = ps5.tile([128, en], F32, tag="ps5")
                        for kt in range(CT):
                            nc.tensor.matmul(
                                pv, convv[:, kt, jt * 128:(jt + 1) * 128],
                                wv[kt][:, e0:e0 + en],
                                start=(kt == 0), stop=(kt == CT - 1))
                        # scatter heads into the 65-strided V_aug layout
                        h0 = e0 // D
                        nh = en // D
                        dstv = vaug[:, jt, h0 * 128:(h0 + nh) * 128]
                        dstv = dstv.rearrange("p (h d) -> p h d", d=128)[:, :, 0:D]
                        nc.vector.tensor_copy(
                            dstv, pv.rearrange("p (h d) -> p h d", d=D))
                nc.sync.dma_start(
                    v_out[b],
                    vaug.rearrange("p j (h d) -> p j h d", d=128)[:, :, :, 0:D])

                # ================= cls-key scores for all heads =================
                kc = kclsp.tile([128, CT * HEADS], F16)
                nc.sync.dma_start(kc, kcls[b])
                kc_v = kc.rearrange("p (t h) -> p t h", t=CT)
                pcls = ps10.tile([12, NPIX], F32, tag="ps10")
                for ch in range(2):
                    for kt in range(CT):
                        nc.tensor.matmul(
                            pcls[:, ch * 512:(ch + 1) * 512], kc_v[:, kt, :],
                            qt[:, kt, ch * 512:(ch + 1) * 512],
                            start=(kt == 0), stop=(kt == CT - 1))
                ec = eclsp.tile([12, NPIX], F16)
                nc.scalar.activation(ec, pcls, mybir.ActivationFunctionType.Exp,
                                     scale=SM_SCALE)
                nc.sync.dma_start(ecls[b], ec)

                # ================= attention =================
                # software-pipelined one head deep (scores(h+1) before ctx(h))
                # with conv(b+1) PSUM groups interleaved two-per-head to fill
                # PE bubbles while ACT grinds the exps
                def issue_scores(h):
                    g, hh = h // 2, h % 2
                    p0, p1 = 64 * hh, 64 * hh + 64
                    et = [None, None]
                    for jt in range(2):
                        pst = ps10.tile([128, NPIX], F32, tag="ps10")
                        for ch in range(2):
                            nc.tensor.matmul(
                                pst[:, ch * 512:(ch + 1) * 512],
                                ktile[p0:p1, g, jt * 128:(jt + 1) * 128],
                                qt[p0:p1, g, ch * 512:(ch + 1) * 512],
                                start=True, stop=True)
                        ex = expp.tile([128, NPIX], F16)
                        nc.scalar.activation(
                            ex, pst, mybir.ActivationFunctionType.Exp,
                            scale=SM_SCALE)
                        et[jt] = ex
                    return et

                def issue_ctx(h, et):
                    st = stage.tile([D + 1, NPIX], F16)
                    for ch in range(2):
                        pc = psctx.tile([128, 512], F32, tag="psctx")
                        for jt in range(2):
                            nc.tensor.matmul(
                                pc, vaug[:, jt, h * 128:(h + 1) * 128],
                                et[jt][:, ch * 512:(ch + 1) * 512],
                                start=(jt == 0), stop=(jt == 1))
                        nc.vector.tensor_copy(
                            st[:, ch * 512:(ch + 1) * 512], pc[0:D + 1, :])
                    nc.sync.dma_start(ctxu[b, h], st)

                if b + 1 < NB:
                    tiles_next, gen_next = conv_make(b + 1)

                prev = None
                for h in range(HEADS):
                    next(gen_next, None)
                    et = issue_scores(h)
                    if prev is not None:
                        issue_ctx(h - 1, prev)
                    next(gen_next, None)
                    prev = et
                for _ in gen_next:
                    pass
                gen_next = iter(())
                issue_ctx(HEADS - 1, prev)
            if rep_ctx is not None:
                rep_ctx.__exit__(None, None, None)

    nc.finalize()
    return nc


def _host_prep(inputs):
    x = np.ascontiguousarray(inputs["x"], dtype=np.float32)     # [B, 1025, 768]
    B = x.shape[0]
    prep = {}

    cw = {}
    shift = {}
    for i, p in enumerate(["q", "k", "v"]):
        g = np.asarray(inputs[f"bn_g_{p}"], np.float32)
        be = np.asarray(inputs[f"bn_b_{p}"], np.float32)
        m = np.asarray(inputs[f"bn_m_{p}"], np.float32)
        v = np.asarray(inputs[f"bn_v_{p}"], np.float32)
        s = g / np.sqrt(v + EPS)
        cw[p] = np.asarray(inputs[f"conv_w_{p}"], np.float32)[:, 0] * s[:, None, None]
        shift[p] = be - m * s

    # xpad: [B, 6, 128, 1156] = zero-padded 34x34 CHW image, fp16
    hs = x[:, 1:, :].reshape(B, 32, 32, EMBED)
    xp = np.zeros((B, EMBED, 34, 34), np.float16)
    xp[:, :, 1:33, 1:33] = hs.transpose(0, 3, 1, 2)
    prep["xpad"] = np.ascontiguousarray(
        xp.reshape(B, CT, 128, 34 * 34))

    # d128: diag conv weights [128, ct, cv(q,v), tap, 128] fp16
    dd = np.zeros((128, CT, 2, 9, 128), np.float16)
    rng = np.arange(128)
    for ci, p in enumerate(["q", "v"]):
        w9 = cw[p].reshape(EMBED, 9)                  # [c, tap]
        for ct in range(CT):
            dd[rng, ct, ci, :, rng] = w9[ct * 128:(ct + 1) * 128, :].astype(
                np.float16)
    prep["d128"] = np.ascontiguousarray(dd.reshape(128, -1))

    # wcolk: [128, 6*9] per-partition k-conv tap weights (f32)
    w9k = cw["k"].reshape(EMBED, 9)
    prep["wcolk"] = np.ascontiguousarray(
        w9k.reshape(CT, 128, 9).transpose(1, 0, 2).reshape(128, -1))

    # w_t: [3, 6, 128, 768]: W^T split into k-tiles, fp16
    prep["w_t"] = np.ascontiguousarray(np.stack([
        np.asarray(inputs[f"W_{p}"], np.float32).T.reshape(CT, 128, EMBED)
        for p in ["q", "k", "v"]])).astype(np.float16)

    # shifts [128, 3*6] (BN shifts, f32); biasq [128, 6]
    sh = np.stack([shift[p].reshape(CT, 128) for p in ["q", "k", "v"]])  # [3,6,128]
    prep["shifts"] = np.ascontiguousarray(sh.transpose(2, 0, 1).reshape(128, -1))
    prep["biasq"] = np.ascontiguousarray(
        np.asarray(inputs["b_q"], np.float32).reshape(CT, 128).T)

    # host-exact cls projections
    cls = x[:, 0, :]                                               # [B, 768]
    Wq = np.asarray(inputs["W_q"], np.float32)
    Wk = np.asarray(inputs["W_k"], np.float32)
    Wv = np.asarray(inputs["W_v"], np.float32)
    b_k = np.asarray(inputs["b_k"], np.float32)
    b_v = np.asarray(inputs["b_v"], np.float32)
    prep["b_k"] = b_k
    prep["b_v"] = b_v
    prep["q_cls"] = cls @ Wq.T + np.asarray(inputs["b_q"], np.float32)
    k_cls = cls @ Wk.T + b_k
    prep["k_cls"] = k_cls
    prep["v_cls"] = cls @ Wv.T + b_v

    # kcls stuffed block lhsT: [B, 128, 6*12]. The device K rows have no
    # k-bias (cancels in softmax), so remove it from the cls key too.
    k_cls_adj = k_cls - b_k
    kc = np.zeros((B, CT, 128, HEADS), np.float16)
    crange = np.arange(EMBED)
    hofc = crange // D                                             # head of channel
    for b in range(B):
        kc[b, crange // 128, crange % 128, hofc] = k_cls_adj[b].astype(np.float16)
    prep["kcls"] = np.ascontiguousarray(kc.transpose(0, 2, 1, 3).reshape(B, 128, -1))
    return prep


def kernel(**inputs) -> np.ndarray:
    global last_results
    x = np.asarray(inputs["x"], np.float32)
    B = x.shape[0]
    assert B == B_TOTAL, f"kernel hardcoded for B={B_TOTAL}, got {B}"

    prep = _host_prep(inputs)
    nc = _build_program()

    in_maps = []
    for c in range(NCORES):
        sl = slice(c * NB, (c + 1) * NB)
        in_maps.append({
            "xpad": prep["xpad"][sl],
            "d128": prep["d128"],
            "w_t": prep["w_t"],
            "wcolk": prep["wcolk"],
            "shifts": prep["shifts"],
            "biasq": prep["biasq"],
            "kcls": prep["kcls"][sl],
        })

    res = run_bass_kernel_spmd(nc, in_maps, core_ids=list(range(NCORES)))
    last_results = res

    # ---- gather + host combine ----
    ctxu = np.concatenate([r["ctxu"] for r in res.results]).astype(
        np.float32)                                                # [B,12,65,1024]
    ecls = np.concatenate([r["ecls"] for r in res.results]).astype(
        np.float32)                                                # [B,12,1024]
    kto = np.concatenate([r["kt_out"] for r in res.results])       # [B,128,6,256]
    vo = np.concatenate([r["v_out"] for r in res.results])         # [B,128,2,768]

    b_k = prep["b_k"]
    b_v = prep["b_v"]
    # K_conv [B, 256, 768] (device rows lack k-bias; restore for cls row)
    k_conv = kto.astype(np.float32).transpose(0, 3, 2, 1).reshape(B, NKV, EMBED) \
        + b_k
    # V rows [B, 256, 768] (device rows lack v-bias; restore for cls row)
    v5 = vo.astype(np.float32).reshape(B, 128, 2, HEADS, D)        # [B,128,2,12,64]
    v_conv = v5.transpose(0, 2, 1, 3, 4).reshape(B, NKV, EMBED) + b_v

    # vch: bias-less cls V for the device-side merge term
    v_cls = prep["v_cls"]                                          # [B, 768]
    vch = (v_cls - b_v).reshape(B, HEADS, D)
    den = ctxu[:, :, D, :] + ecls                                  # [B,12,1024]
    ctx_pix = (ctxu[:, :, :D, :] + ecls[:, :, None, :] * vch[:, :, :, None]) \
        / den[:, :, None, :]                                       # [B,12,64,1024]
    out = np.empty((B, 1 + NPIX, EMBED), np.float32)
    out[:, 1:, :] = ctx_pix.transpose(0, 3, 1, 2).reshape(B, NPIX, EMBED) \
        + b_v                                                      # restore v-bias

    # cls-query row on host (exact fp32)
    k_all = np.concatenate([prep["k_cls"][:, None, :], k_conv], axis=1)  # [B,257,768]
    v_all = np.concatenate([v_cls[:, None, :], v_conv], axis=1)          # [B,257,768]
    qc = prep["q_cls"].reshape(B, HEADS, D)                              # [B,12,64]
    kh = k_all.reshape(B, 257, HEADS, D)
    vh = v_all.reshape(B, 257, HEADS, D)
    s = np.einsum("bhd,bjhd->bhj", qc, kh) * SM_SCALE
    s -= s.max(axis=2, keepdims=True)
    e = np.exp(s)
    p = e / e.sum(axis=2, keepdims=True)
    ctx0 = np.einsum("bhj,bjhd->bhd", p, vh)                             # [B,12,64]
    out[:, 0, :] = ctx0.reshape(B, EMBED)
    return out
